# revision 9
# baseline (speedup 1.0000x reference)
"""Trainium2 Bass kernel for nn_NodeEdgeAttBlock (gnn_message_passing).

Sharding: 8 cores, each owns 64 query rows (bs=2 x n=256 -> 512 rows / 8).
Core c: batch b = c // 4, query offset qoff = (c % 4) * 64.

Device layout: channels on partitions.  Per query i (c split into 2 chunks):
  aT_i  [e=128, j=256]   (host pre-transposes a)
  a1T = Wmul^T aT ; a2T = Wadd^T aT        [c-chunk=128p, j=256]  (PE)
  t1  = a1T * qTs[:,i] + qb[:,i]           (tensor_scalar / activation)
  t2  = t1 * kT                            (TT bf16)
  y   = t2 + a2T                           (TT, psum read)
  e   = exp(y), s = rowsum(e)              (ACT Exp + accum)
  wv  = rowsum(e * vT)                     (tensor_tensor_reduce)
  ahatT = Wao^T y + bao2                   (PE + biased evac; host transposes)
  xhatT = Wxo^T (wv / s) + bxo             (PE, once per core)

Host folds: Wq *= 1/sqrt(F) (and bq), bmul1 = bmul + 1,
bao2 = bao + badd @ Wao (softmax is invariant to the per-channel badd, so
badd drops out of the attention path entirely).
"""

import math
from contextlib import ExitStack

import numpy as np
import ml_dtypes

import concourse.bass as bass
import concourse.bacc as bacc
import concourse.tile as tile
from concourse import mybir
from concourse.bass_utils import run_bass_kernel_spmd

F32 = mybir.dt.float32
BF16 = mybir.dt.bfloat16
ALU = mybir.AluOpType
ACTF = mybir.ActivationFunctionType

BS, N, NH_N, NH_E = 2, 256, 256, 128
N_HEAD = 8
NH_F = NH_N // N_HEAD  # 32
NCORES = 8
NQ = (BS * N) // NCORES  # 64 queries per core
QB = 2                   # queries per psum group
NG = NQ // QB

_cache = {}


def _build_nc():
    nc = bacc.Bacc("TRN2", target_bir_lowering=False, debug=False)

    aT_d = nc.dram_tensor("at_blk", [NQ, NH_E, N], F32, kind="ExternalInput")
    xT_d = nc.dram_tensor("xt", [NH_N, N], F32, kind="ExternalInput")
    xTq_d = nc.dram_tensor("xtq", [NH_N, NQ], F32, kind="ExternalInput")
    wk_d = nc.dram_tensor("wk", [NH_N, NH_N], F32, kind="ExternalInput")
    wv_d = nc.dram_tensor("wvv", [NH_N, NH_N], F32, kind="ExternalInput")
    wq_d = nc.dram_tensor("wq_s", [NH_N, NH_N], F32, kind="ExternalInput")
    bk_d = nc.dram_tensor("bk", [NH_N], F32, kind="ExternalInput")
    bv_d = nc.dram_tensor("bvv", [NH_N], F32, kind="ExternalInput")
    bq_d = nc.dram_tensor("bq_s", [NH_N], F32, kind="ExternalInput")
    wmul_d = nc.dram_tensor("wmul", [NH_E, NH_N], F32, kind="ExternalInput")
    wadd_d = nc.dram_tensor("wadd", [NH_E, NH_N], F32, kind="ExternalInput")
    bmul1_d = nc.dram_tensor("bmul1", [NH_N], F32, kind="ExternalInput")
    wao_d = nc.dram_tensor("wao", [NH_N, NH_E], BF16, kind="ExternalInput")
    bao2_d = nc.dram_tensor("bao2", [NH_E], F32, kind="ExternalInput")
    wxo_d = nc.dram_tensor("wxo", [NH_N, NH_N], F32, kind="ExternalInput")
    bxo_d = nc.dram_tensor("bxo", [NH_N], F32, kind="ExternalInput")

    ahatT_d = nc.dram_tensor("ahatt", [NQ, NH_E, N], F32, kind="ExternalOutput")
    xhatT_d = nc.dram_tensor("xhatt", [2, 128, NQ], F32, kind="ExternalOutput")

    with tile.TileContext(nc) as tc, ExitStack() as ctx:
        const = ctx.enter_context(tc.tile_pool(name="const", bufs=1))

        _n = [0]

        def load(ap, shape, dt=F32):
            _n[0] += 1
            t = const.tile(shape, dt, tag=f"w{_n[0]}", name=f"w{_n[0]}")
            nc.sync.dma_start(t[:], ap)
            return t

        xT = [load(xT_d[m * 128:(m + 1) * 128, :], [128, N]) for m in range(2)]
        xTq = [load(xTq_d[m * 128:(m + 1) * 128, :], [128, NQ]) for m in range(2)]
        wk = [load(wk_d[m * 128:(m + 1) * 128, :], [128, NH_N]) for m in range(2)]
        wv_ = [load(wv_d[m * 128:(m + 1) * 128, :], [128, NH_N]) for m in range(2)]
        wq = [load(wq_d[m * 128:(m + 1) * 128, :], [128, NH_N]) for m in range(2)]
        wmul = load(wmul_d[:, :], [NH_E, NH_N])
        wadd = load(wadd_d[:, :], [NH_E, NH_N])
        wao = [load(wao_d[c * 128:(c + 1) * 128, :], [128, NH_E], dt=BF16) for c in range(2)]
        wxo = [load(wxo_d[c * 128:(c + 1) * 128, :], [128, NH_N]) for c in range(2)]
        bk = [load(bk_d[c * 128:(c + 1) * 128], [128, 1]) for c in range(2)]
        bv_ = [load(bv_d[c * 128:(c + 1) * 128], [128, 1]) for c in range(2)]
        bq = [load(bq_d[c * 128:(c + 1) * 128], [128, 1]) for c in range(2)]
        bmul1 = [load(bmul1_d[c * 128:(c + 1) * 128], [128, 1]) for c in range(2)]
        bao2 = load(bao2_d[:], [128, 1])
        bxo = [load(bxo_d[m * 128:(m + 1) * 128], [128, 1]) for m in range(2)]

        kT = const.tile([128, 2 * N], BF16, tag="kT", name="kT")
        vT = const.tile([128, 2 * N], BF16, tag="vT", name="vT")
        qTs = [const.tile([128, NQ], F32, tag=f"qTs{c}", name=f"qTs{c}") for c in range(2)]
        qb = [const.tile([128, NQ], F32, tag=f"qb{c}", name=f"qb{c}") for c in range(2)]
        sacc = const.tile([128, 2 * NQ], F32, tag="sacc", name="sacc")
        wvu = const.tile([128, 2 * NQ], F32, tag="wvu", name="wvu")

        # ---- prelude (own psum pool, freed before main loop) ----
        with tc.tile_pool(name="ppre", bufs=2, space=bass.MemorySpace.PSUM) as ppre:
            for ci in range(2):
                cs = slice(ci * 128, (ci + 1) * 128)
                pk = ppre.tile([128, N], F32, tag="pk", name="pk")
                nc.tensor.matmul(pk[:], wk[0][:, cs], xT[0][:], start=True, stop=False)
                nc.tensor.matmul(pk[:], wk[1][:, cs], xT[1][:], start=False, stop=True)
                nc.scalar.activation(kT[:, ci * N:(ci + 1) * N], pk[:], ACTF.Identity,
                                     bias=bk[ci][:], scale=1.0)
                pv = ppre.tile([128, N], F32, tag="pv", name="pv")
                nc.tensor.matmul(pv[:], wv_[0][:, cs], xT[0][:], start=True, stop=False)
                nc.tensor.matmul(pv[:], wv_[1][:, cs], xT[1][:], start=False, stop=True)
                nc.scalar.activation(vT[:, ci * N:(ci + 1) * N], pv[:], ACTF.Identity,
                                     bias=bv_[ci][:], scale=1.0)
                pq = ppre.tile([128, NQ], F32, tag="pq", name="pq")
                nc.tensor.matmul(pq[:], wq[0][:, cs], xTq[0][:], start=True, stop=False)
                nc.tensor.matmul(pq[:], wq[1][:, cs], xTq[1][:], start=False, stop=True)
                nc.scalar.activation(qTs[ci][:], pq[:], ACTF.Identity, bias=bq[ci][:],
                                     scale=1.0)
                nc.vector.tensor_scalar(qb[ci][:], qTs[ci][:], bmul1[ci][:], None,
                                        op0=ALU.mult)

        # kT/vT replicated QB times: [128, (ci, q, j)] so group ops batch over q
        kTr = const.tile([128, 2 * QB * N], BF16, tag="kTr", name="kTr")
        vTr = const.tile([128, 2 * QB * N], BF16, tag="vTr", name="vTr")
        for ci in range(2):
            for ql in range(QB):
                o = (ci * QB + ql) * N
                nc.vector.tensor_copy(kTr[:, o:o + N], kT[:, ci * N:(ci + 1) * N])
                nc.vector.tensor_copy(vTr[:, o:o + N], vT[:, ci * N:(ci + 1) * N])

        # ---- main loop ----
        pa = ctx.enter_context(tc.tile_pool(name="pa", bufs=4))
        pt = ctx.enter_context(tc.tile_pool(name="pt", bufs=2))
        pe_ = ctx.enter_context(tc.tile_pool(name="pe", bufs=2))
        pout = ctx.enter_context(tc.tile_pool(name="pout", bufs=3))
        pscr = ctx.enter_context(tc.tile_pool(name="pscr", bufs=2))
        ps1 = ctx.enter_context(tc.tile_pool(name="ps1", bufs=2, space=bass.MemorySpace.PSUM))
        ps2 = ctx.enter_context(tc.tile_pool(name="ps2", bufs=1, space=bass.MemorySpace.PSUM))
        ps3 = ctx.enter_context(tc.tile_pool(name="ps3", bufs=1, space=bass.MemorySpace.PSUM))

        for g in range(NG):
            q0 = g * QB
            aT = pa.tile([128, QB, N], F32, tag="aT", name="aT")
            nc.sync.dma_start(aT[:], aT_d[q0:q0 + QB, :, :].rearrange("q e j -> e q j"))

            a1 = ps1.tile([128, 2 * QB * N], F32, tag="a1", name="a1")
            a2 = ps2.tile([128, 2 * QB * N], F32, tag="a2", name="a2")
            for ci in range(2):
                cs = slice(ci * 128, (ci + 1) * 128)
                o = slice(ci * QB * N, (ci + 1) * QB * N)
                nc.tensor.matmul(a1[:, o], wmul[:, cs], aT[:].rearrange("p q j -> p (q j)"), start=True, stop=True)
                nc.tensor.matmul(a2[:, o], wadd[:, cs], aT[:].rearrange("p q j -> p (q j)"), start=True, stop=True)

            t1 = pt.tile([128, 2 * QB * N], BF16, tag="t1", name="t1")
            for ql in range(QB):
                q = q0 + ql
                o0 = slice(ql * N, (ql + 1) * N)
                nc.vector.tensor_scalar(t1[:, o0], a1[:, o0], qTs[0][:, q:q + 1],
                                        qb[0][:, q:q + 1], op0=ALU.mult, op1=ALU.add)
                o1 = slice(QB * N + ql * N, QB * N + (ql + 1) * N)
                nc.scalar.activation(t1[:, o1], a1[:, o1], ACTF.Identity,
                                     bias=qb[1][:, q:q + 1], scale=qTs[1][:, q:q + 1])

            t2 = pt.tile([128, 2 * QB * N], BF16, tag="t2", name="t2")
            nc.vector.tensor_tensor(t2[:], t1[:], kTr[:], op=ALU.mult)
            y = pt.tile([128, 2 * QB * N], BF16, tag="y", name="y")
            nc.vector.tensor_tensor(y[:], t2[:], a2[:], op=ALU.add)

            e = pe_.tile([128, 2 * QB * N], BF16, tag="e", name="e")
            nc.scalar.activation(e[:], y[:], ACTF.Exp)

            sdst = sacc[:].rearrange("p (c q) -> p c q", c=2)[:, :, q0:q0 + QB]
            nc.vector.tensor_reduce(
                sdst, e[:].rearrange("p (cq j) -> p cq j", j=N),
                axis=mybir.AxisListType.X, op=ALU.add)

            ev = pscr.tile([128, 2 * QB * N], BF16, tag="ev", name="ev")
            nc.vector.tensor_tensor(ev[:], e[:], vTr[:], op=ALU.mult)
            wdst = wvu[:].rearrange("p (c q) -> p c q", c=2)[:, :, q0:q0 + QB]
            nc.vector.tensor_reduce(
                wdst, ev[:].rearrange("p (cq j) -> p cq j", j=N),
                axis=mybir.AxisListType.X, op=ALU.add)

            ah = ps3.tile([128, QB * N], F32, tag="ah", name="ah")
            for ci in range(2):
                o = slice(ci * QB * N, (ci + 1) * QB * N)
                nc.tensor.matmul(ah[:], wao[ci][:], y[:, o], start=(ci == 0),
                                 stop=(ci == 1))
            aho = pout.tile([128, QB * N], F32, tag="aho", name="aho")
            nc.vector.tensor_scalar(aho[:], ah[:], bao2[:], None, op0=ALU.add)
            nc.sync.dma_start(
                ahatT_d[q0:q0 + QB, :, :].rearrange("q e j -> e q j"),
                aho[:].rearrange("p (q j) -> p q j", q=QB))

        # ---- epilogue ----
        wvn = [const.tile([128, NQ], F32, tag=f"wvn{c}", name=f"wvn{c}") for c in range(2)]
        rsa = const.tile([128, 2 * NQ], F32, tag="rsa", name="rsa")
        nc.vector.reciprocal(rsa[:], sacc[:])
        for ci in range(2):
            cs = slice(ci * NQ, (ci + 1) * NQ)
            nc.vector.tensor_tensor(wvn[ci][:], wvu[:, cs], rsa[:, cs], op=ALU.mult)
        for mi in range(2):
            ms = slice(mi * 128, (mi + 1) * 128)
            px = ps3.tile([128, NQ], F32, tag="px", name="px")
            nc.tensor.matmul(px[:], wxo[0][:, ms], wvn[0][:], start=True, stop=False)
            nc.tensor.matmul(px[:], wxo[1][:, ms], wvn[1][:], start=False, stop=True)
            xo = const.tile([128, NQ], F32, tag=f"xo{mi}", name=f"xo{mi}")
            nc.scalar.activation(xo[:], px[:], ACTF.Identity, bias=bxo[mi][:], scale=1.0)
            nc.sync.dma_start(xhatT_d[mi, :, :], xo[:])

    nc.compile()
    return nc


def _prep_inputs(x, a, Wq, bq, Wk, bk, Wv, bv, Wmul, bmul, Wadd, badd, Wxo, bxo,
                 Wao, bao):
    s = 1.0 / math.sqrt(NH_F)
    f32 = lambda v: np.ascontiguousarray(np.asarray(v), dtype=np.float32)
    Wao_np = np.asarray(Wao, np.float32)
    shared = {
        "wk": f32(Wk), "bk": f32(bk),
        "wvv": f32(Wv), "bvv": f32(bv),
        "wq_s": f32(np.asarray(Wq) * s), "bq_s": f32(np.asarray(bq) * s),
        "wmul": f32(Wmul), "bmul1": f32(np.asarray(bmul) + 1.0),
        "wadd": f32(Wadd),
        "wao": np.ascontiguousarray(Wao_np.astype(ml_dtypes.bfloat16)), "bao2": f32(np.asarray(bao) + np.asarray(badd, np.float32) @ Wao_np),
        "wxo": f32(Wxo), "bxo": f32(bxo),
    }
    x = np.asarray(x, np.float32)
    a = np.asarray(a, np.float32)
    in_maps = []
    for c in range(NCORES):
        b = c // 4
        qoff = (c % 4) * NQ
        xT = f32(x[b].T)
        m = dict(shared)
        m["at_blk"] = f32(np.transpose(a[b, qoff:qoff + NQ], (0, 2, 1)))
        m["xt"] = xT
        m["xtq"] = f32(xT[:, qoff:qoff + NQ])
        in_maps.append(m)
    return in_maps


def kernel(**inputs):
    if "nc" not in _cache:
        _cache["nc"] = _build_nc()
    nc = _cache["nc"]
    in_maps = _prep_inputs(**inputs)
    res = run_bass_kernel_spmd(nc, in_maps, core_ids=list(range(NCORES)))
    xhat = np.zeros((BS, N, NH_N), np.float32)
    ahat = np.zeros((BS, N, N, NH_E), np.float32)
    for c in range(NCORES):
        b = c // 4
        qoff = (c % 4) * NQ
        r = res.results[c]
        ahat[b, qoff:qoff + NQ] = np.transpose(r["ahatt"], (0, 2, 1))
        xh = np.concatenate([r["xhatt"][0], r["xhatt"][1]], axis=0)
        xhat[b, qoff:qoff + NQ] = xh.T
    return xhat, ahat


# revision 13
# speedup vs baseline: 761.1686x; 761.1686x over previous
"""Trainium2 Bass kernel for nn_NodeEdgeAttBlock (gnn_message_passing).

Sharding: 8 cores, each owns 64 query rows (bs=2 x n=256 -> 512 rows / 8).
Core c: batch b = c // 4, query offset qoff = (c % 4) * 64.

Device layout: channels on partitions.  Per query i (c split into 2 chunks):
  aT_i  [e=128, j=256]   (host pre-transposes a)
  a1T = Wmul^T aT ; a2T = Wadd^T aT        [c-chunk=128p, j=256]  (PE)
  t1  = a1T * qTs[:,i] + qb[:,i]           (tensor_scalar / activation)
  t2  = t1 * kT                            (TT bf16)
  y   = t2 + a2T                           (TT, psum read)
  e   = exp(y), s = rowsum(e)              (ACT Exp + accum)
  wv  = rowsum(e * vT)                     (tensor_tensor_reduce)
  ahatT = Wao^T y + bao2                   (PE + biased evac; host transposes)
  xhatT = Wxo^T (wv / s) + bxo             (PE, once per core)

Host folds: Wq *= 1/sqrt(F) (and bq), bmul1 = bmul + 1,
bao2 = bao + badd @ Wao (softmax is invariant to the per-channel badd, so
badd drops out of the attention path entirely).
"""

import math
from contextlib import ExitStack

import numpy as np
import ml_dtypes

import concourse.bass as bass
import concourse.bacc as bacc
import concourse.tile as tile
from concourse import mybir
from concourse.bass_utils import run_bass_kernel_spmd
import jax
from jax.experimental.shard_map import shard_map
from jax.sharding import Mesh, PartitionSpec
from concourse import bass2jax


def make_runner(nc, n_cores):
    bass2jax.install_neuronx_cc_hook()
    partition_name = nc.partition_id_tensor.name if nc.partition_id_tensor else None

    in_names, out_names, out_avals, zero_shapes = [], [], [], []
    for alloc in nc.m.functions[0].allocations:
        if not isinstance(alloc, mybir.MemoryLocationSet):
            continue
        name = alloc.memorylocations[0].name
        if alloc.kind == "ExternalInput":
            if name != partition_name:
                in_names.append(name)
        elif alloc.kind == "ExternalOutput":
            shape = tuple(alloc.tensor_shape)
            dtype = mybir.dt.np(alloc.dtype)
            out_names.append(name)
            out_avals.append(jax.core.ShapedArray(shape, dtype))
            zero_shapes.append((shape, dtype))
    n_params = len(in_names)
    n_outs = len(out_avals)
    all_in_names = list(in_names) + list(out_names)
    if partition_name is not None:
        all_in_names.append(partition_name)

    def _body(*args):
        operands = list(args)
        if partition_name is not None:
            operands.append(bass2jax.partition_id_tensor())
        outs = bass2jax._bass_exec_p.bind(
            *operands,
            out_avals=tuple(out_avals),
            in_names=tuple(all_in_names),
            out_names=tuple(out_names),
            lowering_input_output_aliases=(),
            sim_require_finite=True,
            sim_require_nnan=True,
            nc=nc,
        )
        return tuple(outs)

    devices = jax.devices()[:n_cores]
    mesh = Mesh(np.asarray(devices), ("core",))
    in_specs = (PartitionSpec("core"),) * (n_params + n_outs)
    out_specs = (PartitionSpec("core"),) * n_outs
    donate = tuple(range(n_params, n_params + n_outs))
    sharded = jax.jit(
        shard_map(_body, mesh=mesh, in_specs=in_specs, out_specs=out_specs,
                  check_rep=False),
        donate_argnums=donate, keep_unused=True,
    )

    def concat(in_maps):
        per_core = [[np.asarray(m[name]) for name in in_names] for m in in_maps]
        concat_in = [
            np.concatenate([per_core[c][i] for c in range(n_cores)], axis=0)
            for i in range(n_params)
        ]
        concat_zeros = [
            np.zeros((n_cores * s[0], *s[1:]), dt) for (s, dt) in zero_shapes
        ]
        return concat_in, concat_zeros

    _cache["runner_parts"] = {"fn": sharded, "mesh": mesh, "concat": concat}

    def run(in_maps):
        concat_in, concat_zeros = concat(in_maps)
        out_arrs = sharded(*concat_in, *concat_zeros)
        out_arrs = [np.asarray(o) for o in out_arrs]
        return [
            {name: out_arrs[i].reshape(n_cores, *out_avals[i].shape)[c]
             for i, name in enumerate(out_names)}
            for c in range(n_cores)
        ]

    return run


F32 = mybir.dt.float32
BF16 = mybir.dt.bfloat16
ALU = mybir.AluOpType
ACTF = mybir.ActivationFunctionType

BS, N, NH_N, NH_E = 2, 256, 256, 128
N_HEAD = 8
NH_F = NH_N // N_HEAD  # 32
NCORES = 8
NQ = (BS * N) // NCORES  # 64 queries per core
QB = 2                   # queries per psum group
NG = NQ // QB
QL = 16                  # queries per DMA batch
GPL = QL // QB           # groups per DMA batch

_cache = {}


def _build_nc():
    nc = bacc.Bacc("TRN2", target_bir_lowering=False, debug=False)

    aT_d = nc.dram_tensor("at_blk", [NQ, NH_E, N], BF16, kind="ExternalInput")
    CF = 2 * N + 2 * NQ + 8 * NH_N + 11   # f32 blob cols
    CB = 3 * NH_N                          # bf16 blob cols
    wf_d = nc.dram_tensor("wf", [128, CF], F32, kind="ExternalInput")
    wb_d = nc.dram_tensor("wb", [128, CB], BF16, kind="ExternalInput")

    ahatT_d = nc.dram_tensor("ahatt", [NQ, NH_E, N], F32, kind="ExternalOutput")
    xhatT_d = nc.dram_tensor("xhatt", [2, 128, NQ], F32, kind="ExternalOutput")

    with tile.TileContext(nc) as tc, ExitStack() as ctx:
        const = ctx.enter_context(tc.tile_pool(name="const", bufs=1))

        wf = const.tile([128, CF], F32, tag="wf", name="wf")
        nc.sync.dma_start(wf[:], wf_d[:, :])
        wb = const.tile([128, CB], BF16, tag="wb", name="wb")
        nc.sync.dma_start(wb[:], wb_d[:, :])

        _c = [0]

        def piece(w):
            o = _c[0]
            _c[0] += w
            return wf[:, o:o + w]

        xT = [piece(N) for _ in range(2)]
        xTq = [piece(NQ) for _ in range(2)]
        wk = [piece(NH_N) for _ in range(2)]
        wv_ = [piece(NH_N) for _ in range(2)]
        wq = [piece(NH_N) for _ in range(2)]
        wxo = [piece(NH_N) for _ in range(2)]
        bk = [piece(1) for _ in range(2)]
        bv_ = [piece(1) for _ in range(2)]
        bq = [piece(1) for _ in range(2)]
        bmul1 = [piece(1) for _ in range(2)]
        bao2 = piece(1)
        bxo = [piece(1) for _ in range(2)]
        wmul = wb[:, 0:NH_N]
        wadd = wb[:, NH_N:2 * NH_N]
        wao = [wb[:, 2 * NH_N + c * 128:2 * NH_N + (c + 1) * 128] for c in range(2)]

        kT = const.tile([128, 2 * N], BF16, tag="kT", name="kT")
        vT = const.tile([128, 2 * N], BF16, tag="vT", name="vT")
        qTs = [const.tile([128, NQ], F32, tag=f"qTs{c}", name=f"qTs{c}") for c in range(2)]
        qb = [const.tile([128, NQ], F32, tag=f"qb{c}", name=f"qb{c}") for c in range(2)]
        sacc = const.tile([128, 2 * NQ], F32, tag="sacc", name="sacc")
        wvu = const.tile([128, 2 * NQ], F32, tag="wvu", name="wvu")

        # ---- prelude (own psum pool, freed before main loop) ----
        with tc.tile_pool(name="ppre", bufs=2, space=bass.MemorySpace.PSUM) as ppre:
            for ci in range(2):
                cs = slice(ci * 128, (ci + 1) * 128)
                pk = ppre.tile([128, N], F32, tag="pk", name="pk")
                nc.tensor.matmul(pk[:], wk[0][:, cs], xT[0][:], start=True, stop=False)
                nc.tensor.matmul(pk[:], wk[1][:, cs], xT[1][:], start=False, stop=True)
                nc.scalar.activation(kT[:, ci * N:(ci + 1) * N], pk[:], ACTF.Identity,
                                     bias=bk[ci][:], scale=1.0)
                pv = ppre.tile([128, N], F32, tag="pv", name="pv")
                nc.tensor.matmul(pv[:], wv_[0][:, cs], xT[0][:], start=True, stop=False)
                nc.tensor.matmul(pv[:], wv_[1][:, cs], xT[1][:], start=False, stop=True)
                nc.scalar.activation(vT[:, ci * N:(ci + 1) * N], pv[:], ACTF.Identity,
                                     bias=bv_[ci][:], scale=1.0)
                pq = ppre.tile([128, NQ], F32, tag="pq", name="pq")
                nc.tensor.matmul(pq[:], wq[0][:, cs], xTq[0][:], start=True, stop=False)
                nc.tensor.matmul(pq[:], wq[1][:, cs], xTq[1][:], start=False, stop=True)
                nc.scalar.activation(qTs[ci][:], pq[:], ACTF.Identity, bias=bq[ci][:],
                                     scale=1.0)
                nc.vector.tensor_scalar(qb[ci][:], qTs[ci][:], bmul1[ci][:], None,
                                        op0=ALU.mult)

        # kT/vT replicated QB times: [128, (ci, q, j)] so group ops batch over q
        kTr = const.tile([128, 2 * QB * N], BF16, tag="kTr", name="kTr")
        vTr = const.tile([128, 2 * QB * N], BF16, tag="vTr", name="vTr")
        for ci in range(2):
            for ql in range(QB):
                o = (ci * QB + ql) * N
                nc.vector.tensor_copy(kTr[:, o:o + N], kT[:, ci * N:(ci + 1) * N])
                nc.vector.tensor_copy(vTr[:, o:o + N], vT[:, ci * N:(ci + 1) * N])

        # ---- main loop ----
        pa = ctx.enter_context(tc.tile_pool(name="pa", bufs=2))
        pt = ctx.enter_context(tc.tile_pool(name="pt", bufs=2))
        pe_ = ctx.enter_context(tc.tile_pool(name="pe", bufs=2))
        pout = ctx.enter_context(tc.tile_pool(name="pout", bufs=2))
        pscr = ctx.enter_context(tc.tile_pool(name="pscr", bufs=2))
        ps1 = ctx.enter_context(tc.tile_pool(name="ps1", bufs=2, space=bass.MemorySpace.PSUM))
        ps2 = ctx.enter_context(tc.tile_pool(name="ps2", bufs=1, space=bass.MemorySpace.PSUM))
        ps3 = ctx.enter_context(tc.tile_pool(name="ps3", bufs=1, space=bass.MemorySpace.PSUM))

        for g in range(NG):
            q0 = g * QB
            gl = g % GPL
            if gl == 0:
                aT8 = pa.tile([128, QL, N], BF16, tag="aT8", name="aT8")
                nc.sync.dma_start(
                    aT8[:], aT_d[q0:q0 + QL, :, :].rearrange("q e j -> e q j"))
                ast = pout.tile([128, QL * N], F32, tag="ast", name="ast")
            aT = aT8[:, gl * QB:(gl + 1) * QB, :].rearrange("p q j -> p (q j)")

            a1 = ps1.tile([128, 2 * QB * N], F32, tag="a1", name="a1")
            a2 = ps2.tile([128, 2 * QB * N], F32, tag="a2", name="a2")
            for ci in range(2):
                cs = slice(ci * 128, (ci + 1) * 128)
                o = slice(ci * QB * N, (ci + 1) * QB * N)
                nc.tensor.matmul(a1[:, o], wmul[:, cs], aT, start=True, stop=True)
                nc.tensor.matmul(a2[:, o], wadd[:, cs], aT, start=True, stop=True)

            t1 = pt.tile([128, 2 * QB * N], BF16, tag="t1", name="t1")
            for ql in range(QB):
                q = q0 + ql
                o0 = slice(ql * N, (ql + 1) * N)
                nc.vector.tensor_scalar(t1[:, o0], a1[:, o0], qTs[0][:, q:q + 1],
                                        qb[0][:, q:q + 1], op0=ALU.mult, op1=ALU.add)
                o1 = slice(QB * N + ql * N, QB * N + (ql + 1) * N)
                nc.scalar.activation(t1[:, o1], a1[:, o1], ACTF.Identity,
                                     bias=qb[1][:, q:q + 1], scale=qTs[1][:, q:q + 1])

            t2 = pt.tile([128, 2 * QB * N], BF16, tag="t2", name="t2")
            nc.gpsimd.tensor_tensor(t2[:], t1[:], kTr[:], op=ALU.mult)
            y = pt.tile([128, 2 * QB * N], BF16, tag="y", name="y")
            nc.vector.tensor_tensor(y[:], t2[:], a2[:], op=ALU.add)

            e = pe_.tile([128, 2 * QB * N], BF16, tag="e", name="e")
            nc.scalar.activation(e[:], y[:], ACTF.Exp)

            sdst = sacc[:].rearrange("p (c q) -> p c q", c=2)[:, :, q0:q0 + QB]
            nc.vector.tensor_reduce(
                sdst, e[:].rearrange("p (cq j) -> p cq j", j=N),
                axis=mybir.AxisListType.X, op=ALU.add)

            ev = pscr.tile([128, 2 * QB * N], BF16, tag="ev", name="ev")
            nc.gpsimd.tensor_tensor(ev[:], e[:], vTr[:], op=ALU.mult)
            wdst = wvu[:].rearrange("p (c q) -> p c q", c=2)[:, :, q0:q0 + QB]
            nc.vector.tensor_reduce(
                wdst, ev[:].rearrange("p (cq j) -> p cq j", j=N),
                axis=mybir.AxisListType.X, op=ALU.add)

            ah = ps3.tile([128, QB * N], F32, tag="ah", name="ah")
            for ci in range(2):
                o = slice(ci * QB * N, (ci + 1) * QB * N)
                nc.tensor.matmul(ah[:], wao[ci][:], y[:, o], start=(ci == 0),
                                 stop=(ci == 1))
            nc.vector.tensor_scalar(ast[:, gl * QB * N:(gl + 1) * QB * N], ah[:],
                                    bao2[:], None, op0=ALU.add)
            if gl == GPL - 1:
                qs = (g - gl) * QB
                nc.sync.dma_start(
                    ahatT_d[qs:qs + QL, :, :].rearrange("q e j -> e q j"),
                    ast[:].rearrange("p (q j) -> p q j", q=QL))

        # ---- epilogue ----
        wvn = [const.tile([128, NQ], F32, tag=f"wvn{c}", name=f"wvn{c}") for c in range(2)]
        rsa = const.tile([128, 2 * NQ], F32, tag="rsa", name="rsa")
        nc.vector.reciprocal(rsa[:], sacc[:])
        for ci in range(2):
            cs = slice(ci * NQ, (ci + 1) * NQ)
            nc.vector.tensor_tensor(wvn[ci][:], wvu[:, cs], rsa[:, cs], op=ALU.mult)
        for mi in range(2):
            ms = slice(mi * 128, (mi + 1) * 128)
            px = ps3.tile([128, NQ], F32, tag="px", name="px")
            nc.tensor.matmul(px[:], wxo[0][:, ms], wvn[0][:], start=True, stop=False)
            nc.tensor.matmul(px[:], wxo[1][:, ms], wvn[1][:], start=False, stop=True)
            xo = const.tile([128, NQ], F32, tag=f"xo{mi}", name=f"xo{mi}")
            nc.scalar.activation(xo[:], px[:], ACTF.Identity, bias=bxo[mi][:], scale=1.0)
            nc.sync.dma_start(xhatT_d[mi, :, :], xo[:])

    nc.compile()
    return nc


def _prep_inputs(x, a, Wq, bq, Wk, bk, Wv, bv, Wmul, bmul, Wadd, badd, Wxo, bxo,
                 Wao, bao):
    s = 1.0 / math.sqrt(NH_F)
    f32 = lambda v: np.asarray(v, np.float32)
    Wao_np = f32(Wao)
    bao2 = f32(bao) + f32(badd) @ Wao_np

    def chunks2(w):  # (256, C) -> two [128, C] pieces
        w = f32(w)
        return [w[0:128], w[128:256]]

    def cols2(v):    # (256,) -> two [128, 1] pieces
        v = f32(v)
        return [v[0:128, None], v[128:256, None]]

    wb = np.concatenate([f32(Wmul), f32(Wadd), f32(Wao).reshape(2, 128, NH_E)
                         .transpose(1, 0, 2).reshape(128, 2 * NH_E)], axis=1)
    wb = np.ascontiguousarray(wb.astype(ml_dtypes.bfloat16))

    x = f32(x)
    a = f32(a)
    in_maps = []
    for c in range(NCORES):
        b = c // 4
        qoff = (c % 4) * NQ
        xT = np.ascontiguousarray(x[b].T)
        pieces = (chunks2(xT) + chunks2(xT[:, qoff:qoff + NQ]) + chunks2(Wk)
                  + chunks2(Wv) + chunks2(f32(Wq) * s) + chunks2(Wxo)
                  + cols2(bk) + cols2(bv) + cols2(f32(bq) * s)
                  + cols2(f32(bmul) + 1.0) + [bao2[:, None]] + cols2(bxo))
        wf = np.ascontiguousarray(np.concatenate(pieces, axis=1), dtype=np.float32)
        m = {
            "wf": wf, "wb": wb,
            "at_blk": np.ascontiguousarray(
                np.transpose(a[b, qoff:qoff + NQ], (0, 2, 1)).astype(ml_dtypes.bfloat16)),
        }
        in_maps.append(m)
    return in_maps


def kernel(**inputs):
    if "nc" not in _cache:
        _cache["nc"] = _build_nc()
        _cache["run"] = make_runner(_cache["nc"], NCORES)
    in_maps = _prep_inputs(**inputs)
    res = _cache["run"](in_maps)
    xhat = np.zeros((BS, N, NH_N), np.float32)
    ahat = np.zeros((BS, N, N, NH_E), np.float32)
    for c in range(NCORES):
        b = c // 4
        qoff = (c % 4) * NQ
        r = res[c]
        ahat[b, qoff:qoff + NQ] = np.transpose(r["ahatt"], (0, 2, 1))
        xh = np.concatenate([r["xhatt"][0], r["xhatt"][1]], axis=0)
        xhat[b, qoff:qoff + NQ] = xh.T
    return xhat, ahat
